# revision 39
# baseline (speedup 1.0000x reference)
"""Trainium2 Bass kernel for nn_CE_55937654063537.

Reference computation:
    b1 = conv3x3(x, g_w) + g_b            [B, 2, 512, 512]
    b2 = conv1x1(x, theta_w) + theta_b    [B, 2, 512, 512]
    m  = patch_mean(b1, 7) + patch_mean(b2, 7)   [B, 2, 7, 7]
    out = bilinear_upsample(m, 512, 512)  (half-pixel centers)

Everything is linear, so the kernel never materializes the conv outputs.
patch_mean(feat)[i, j] is (1/(H*W)) * the sum of feat over a rectangle that is
the full map minus <=3 boundary rows/cols.  Those rectangle sums are linear in
(a) the column-sum over h of x and (b) 8 boundary rows of x.

Final dataflow (engine-balanced so nothing exceeds the DMA period):
  loads:   SWDGE (gpsimd) 2 MB chunk DMAs that CAST fp32 -> bf16 on the fly;
           the HBM read side streams at the 16-engine descriptor line rate
           (~430 GB/s), SBUF tiles halve
  colsum:  two pairwise bf16 adds on DVE (2x perf mode) reduce each chunk
           8 -> 2 h-slices; PSUM-chained matmuls against a bf16 channel
           indicator finish the reduction (fp32 PSUM, exact)
  stats:   ONE up-front DMA lands all batches' edge rows in Sall[36, B, 512];
           per-row [total + 8 edge columns] on DVE, tiny fp32 matmuls -> R
  tg:      one merged matmul -> [39, 512] PSUM (co=1 block at partition 32;
           unwritten Et cols 7:32 produce garbage rows nobody reads)
  out:     8 bf16 matmuls/batch (FWL weight loads) against A^T quadrant
           slices (pair-interleaved rows -> 4 KB store descriptors); the two
           256-row halves drain through parallel ACT and DVE wide copies
  stores:  one 512 KB store per (channel, half), split across the sync HWDGE
           ring and the SWDGE queue; consts ride the head of the sync ring
Engine budget per 4 MB batch: DVE ~6 us, PE ~8 us, ACT ~5 us, all under the
~15 us DMA period, so the whole kernel pipelines behind the DMA stream.

Data parallel over batch: 8 cores x 4 batches each; params replicated.
"""
import numpy as np

H = W = 512
K = 7
CIN = 4
CO = 2
BLOC = 4    # batches per core
NCORES = 8

_PROG = None          # cached Bass program (weight-independent; weights are inputs)
N_REPS = 1
NCHUNK = 2            # load DMAs per 4 MB batch
XBUFS = 8             # chunk-tile buffering depth (1 MB each, bf16)
TRACE = False
LAST_EXEC_NS = None
LAST_TRACE_PATH = None

# f32r const tensor [7, 512]: la = L@A^T.  The separate bf16 atr tensor holds
# the four out-expansion lhsT slices: slice (g, tt) at 256g+128tt has
# [i, p] = A[256g + 2p + tt, i] (pair-interleaved rows -> 4 KB store descs)
C7_LA = 0
C7W = 512
# packed fp32 const tensor [36, CFW]: blk | biasrow | biaspat
CF_BLK = 0
CF_BIAS = 42
CF_BPAT = 56
CFW = 63


# ---------------------------------------------------------------------------
# host-side constant builders (all tiny, derived from conv weights)
# ---------------------------------------------------------------------------

def resize_mat(in_size, out_size):
    """Bilinear (half-pixel, edge-normalized) interpolation matrix [out, in],
    matching jax.image.resize(method='bilinear') for upsampling."""
    inv_scale = in_size / out_size
    sample_f = (np.arange(out_size) + 0.5) * inv_scale - 0.5
    xw = np.abs(sample_f[None, :] - np.arange(in_size)[:, None])
    weights = np.maximum(0, 1 - xw)
    total = weights.sum(axis=0, keepdims=True)
    return (weights / total).T.astype(np.float32)  # [out, in]


def build_lhsTR(g_w, g_b, theta_w, theta_b):
    """Phase-2 weight blocks (per batch; identical for every b).

    Returns (blk [4, 3, 9, 14], bias [1, 14]):
      blk[ci, dw, q, col]: coefficient of stats row q of channel ci
        (q: 0=colsum over h, 1..4=x rows 0..3, 5..8=x rows 508..511)
        in output row col = co*7 + i -> R[co, i][w] under w-shift dw.
      bias[0, col]: additive constant (applies to every w of R[col]).
    """
    gw = g_w.astype(np.float64)
    gb = g_b.astype(np.float64)
    tw = theta_w.astype(np.float64)[:, :, 0, 0]
    tb = theta_b.astype(np.float64)
    blk = np.zeros((CIN, 3, 9, 14), dtype=np.float64)
    bias = np.zeros((1, 14), dtype=np.float64)

    def add_F(col, co, dw, sign):
        for ci in range(CIN):
            blk[ci, dw, 0, col] += sign * gw[co, ci, :, dw].sum()
            blk[ci, dw, 1, col] += -sign * gw[co, ci, 2, dw]   # x row 0
            blk[ci, dw, 8, col] += -sign * gw[co, ci, 0, dw]   # x row 511
            if dw == 1:
                blk[ci, dw, 0, col] += sign * tw[co, ci]
        if dw == 1:
            bias[0, col] += sign * H * (gb[co] + tb[co])

    def add_bd(col, co, r, dw, sign):
        for ci in range(CIN):
            for dh in range(3):
                hr = r + dh - 1
                if 0 <= hr < H:
                    q = 1 + hr if hr <= 3 else 5 + (hr - (H - 4))
                    blk[ci, dw, q, col] += sign * gw[co, ci, dh, dw]
            if dw == 1:
                q = 1 + r if r <= 3 else 5 + (r - (H - 4))
                blk[ci, dw, q, col] += sign * tw[co, ci]
        if dw == 1:
            bias[0, col] += sign * (gb[co] + tb[co])

    for co in range(CO):
        for i in range(K):
            col = co * 7 + i
            for dw in range(3):
                add_F(col, co, dw, 1.0)
                if i < 3:
                    for r in range(H - 3 + i, H):
                        add_bd(col, co, r, dw, -1.0)
                elif i > 3:
                    for r in range(0, i - 3):
                        add_bd(col, co, r, dw, -1.0)
    return blk.astype(np.float32), bias.astype(np.float32)


def build_L():
    """Phase-3 lhsT [7, 7] (includes the 1/(H*W) patch-mean scale)."""
    L = np.zeros((7, 7), dtype=np.float64)
    L[0, :] = 1.0
    for j in range(3):            # j=0,1,2: subtract tail elements w >= 509+j
        for e in range(3 + j, 6):
            L[1 + e, j] = -1.0    # e=3,4,5 -> rows 4..6
    for j in range(4, 7):         # j=4,5,6: subtract head elements w < j-3
        for e in range(0, j - 3):
            L[1 + e, j] = -1.0    # e=0,1,2 -> rows 1..3
    return (L / (H * W)).astype(np.float32)


def build_consts(g_w, g_b, theta_w, theta_b):
    import ml_dtypes
    blk, biasrow = build_lhsTR(g_w, g_b, theta_w, theta_b)
    A = resize_mat(K, H)          # [512, 7]
    biaspat = np.ones((1, 7), dtype=np.float32)
    biaspat[0, 0] = float(W)      # total-sum column gets bias once per w
    la = build_L().astype(np.float64) @ A.astype(np.float64).T     # [7, 512]
    # stats live on 36 partitions: q*4 + ci (q: 0=colsum, 1..8=edge rows)
    blk4 = np.ascontiguousarray(blk.transpose(2, 0, 1, 3).reshape(36, 3, 14))

    ind = np.zeros((128, CIN), dtype=np.float32)
    for c in range(CIN):          # channel indicator (0/1: exact in bf16)
        ind[32 * c:32 * (c + 1), c] = 1.0

    c7 = np.zeros((7, C7W), dtype=np.float32)
    c7[:, C7_LA:C7_LA + 512] = la.astype(np.float32)
    Ar = A.reshape(2, 128, 2, K)              # [g, p, tt, i]; row = 256g+2p+tt
    atr = Ar.transpose(3, 0, 2, 1).reshape(K, 512)

    cf = np.zeros((36, CFW), dtype=np.float32)
    cf[:, CF_BLK:CF_BLK + 42] = blk4.reshape(36, 42)
    cf[0:1, CF_BIAS:CF_BIAS + 14] = biasrow
    cf[0:1, CF_BPAT:CF_BPAT + 7] = biaspat
    return {"ind": ind.astype(ml_dtypes.bfloat16), "c7": c7, "cf": cf,
            "atr": atr.astype(ml_dtypes.bfloat16)}


# ---------------------------------------------------------------------------
# device program
# ---------------------------------------------------------------------------

def build_program():
    import concourse.bass as bass
    import concourse.bacc as bacc
    import concourse.tile as tile
    from concourse import mybir

    f32 = mybir.dt.float32
    f32r = mybir.dt.float32r
    bf16 = mybir.dt.bfloat16
    nc = bacc.Bacc(None, target_bir_lowering=False, enable_partition_id=False)

    xs = nc.dram_tensor("xs", [BLOC, 128, 16, W], f32, kind="ExternalInput")
    xe_d = nc.dram_tensor("xe", [BLOC, 2, 4, CIN, W], f32r, kind="ExternalInput")
    ind_d = nc.dram_tensor("ind", [128, CIN], bf16, kind="ExternalInput")
    c7_d = nc.dram_tensor("c7", [7, C7W], f32r, kind="ExternalInput")
    atr_d = nc.dram_tensor("atr", [7, 512], bf16, kind="ExternalInput")
    cf_d = nc.dram_tensor("cf", [36, CFW], f32, kind="ExternalInput")
    y = nc.dram_tensor("y", [BLOC, CO, H, W], f32, kind="ExternalOutput")

    TPC = 16 // NCHUNK            # t-tiles per load chunk

    with tile.TileContext(nc) as tc:
        with (
            tc.tile_pool(name="consts", bufs=1) as consts,
            tc.tile_pool(name="xpool", bufs=XBUFS) as xpool,
            tc.tile_pool(name="hpool", bufs=4) as hpool,
            tc.tile_pool(name="spool", bufs=1) as spool,
            tc.tile_pool(name="vpool", bufs=2) as vpool,
            tc.tile_pool(name="etp", bufs=2) as etp,
            tc.tile_pool(name="tgpool", bufs=2) as tgpool,
            tc.tile_pool(name="opool", bufs=4) as opool,
            tc.tile_pool(name="pstats", bufs=2, space="PSUM") as pstats,
            tc.tile_pool(name="pr", bufs=1, space="PSUM") as pr,
            tc.tile_pool(name="ptg", bufs=1, space="PSUM") as ptg,
            tc.tile_pool(name="poc", bufs=2, space="PSUM") as poc,
        ):
            # consts + edge rows at the HEAD of the sync HWDGE ring (FIFO:
            # they land before the ring's stores and contend with nothing)
            c_ind = consts.tile([128, CIN], bf16)
            nc.sync.dma_start(out=c_ind, in_=ind_d[:, :])
            c_7 = consts.tile([7, C7W], f32r)
            nc.sync.dma_start(out=c_7, in_=c7_d[:, :])
            c_atr = consts.tile([7, 512], bf16)
            nc.sync.dma_start(out=c_atr, in_=atr_d[:, :])
            c_f = consts.tile([36, CFW], f32)
            nc.sync.dma_start(out=c_f, in_=cf_d[:, :])
            c_la = c_7[:, C7_LA:C7_LA + 512]
            c_bias = c_f[0:1, CF_BIAS:CF_BIAS + 14]
            c_bpat = c_f[0:1, CF_BPAT:CF_BPAT + 7]

            def emit_once():
                # ONE DMA lands every batch's 8 edge rows: Sall[q, b, w]
                # (q: 0..3 colsum per channel, 4..35 edge rows x channel)
                Sall = spool.tile([36, BLOC, W], f32r, tag="Sall")
                nc.sync.dma_start(
                    out=Sall[4:36, :, :],
                    in_=xe_d.rearrange("b e r c w -> (e r c) b w"),
                )

                def load_colsum(b):
                    # SWDGE cast-loads (fp32 -> bf16); two DVE pairwise adds
                    # (2x bf16) reduce each 2 MB chunk 8 -> 2 slices; the
                    # PSUM-chained indicator matmuls finish the reduction
                    st = pstats.tile([CIN, W], f32, tag="st")
                    for k in range(NCHUNK):
                        xt = xpool.tile([128, TPC, W], bf16, tag="xt")
                        nc.gpsimd.dma_start(
                            out=xt, in_=xs[b, :, k * TPC:(k + 1) * TPC, :])
                        h4 = hpool.tile([128, 4, W], bf16, tag="h4")
                        nc.vector.tensor_add(h4, xt[:, 0:4, :], xt[:, 4:8, :])
                        h2 = hpool.tile([128, 2, W], bf16, tag="h2")
                        nc.vector.tensor_add(h2, h4[:, 0:2, :], h4[:, 2:4, :])
                        for j in range(2):
                            nc.tensor.matmul(
                                st, c_ind, h2[:, j, :],
                                start=(k == 0 and j == 0),
                                stop=(k == NCHUNK - 1 and j == 1))
                    return st

                def stage_v(b, st):
                    # stats -> V = [T | edge columns], three 7-col groups,
                    # one per w-shift dw of the 3x3 conv
                    S = Sall[:, b, :]
                    nc.scalar.copy(S[0:CIN, :], st)
                    V = vpool.tile([36, 21], f32, tag="V")
                    nc.vector.reduce_sum(V[:, 7:8], S, axis=mybir.AxisListType.X)
                    edges = bass.AP(           # S columns {0,1,2, 509,510,511}
                        tensor=S.tensor, offset=S.offset,
                        ap=[S.ap[0], [509, 2], [1, 3]],
                    )
                    nc.vector.tensor_copy(
                        V[:, 8:14].rearrange("q (g e) -> q g e", g=2), edges)
                    nc.vector.tensor_sub(V[:, 0:1], V[:, 7:8], V[:, 13:14])
                    nc.vector.memset(V[:, 1:2], 0.0)
                    nc.vector.tensor_copy(V[:, 2:4], V[:, 8:10])
                    nc.vector.tensor_copy(V[:, 4:7], S[:, 508:511])
                    nc.vector.tensor_sub(V[:, 14:15], V[:, 7:8], V[:, 8:9])
                    nc.vector.tensor_copy(V[:, 15:18], S[:, 1:4])
                    nc.vector.tensor_copy(V[:, 18:20], V[:, 12:14])
                    nc.vector.memset(V[:, 20:21], 0.0)
                    return V

                def stage_r(b, V):
                    # Et = R^T; co=1 block at cols 32:39 so the merged tg
                    # matmul's output reads back 32-aligned
                    Rt = pr.tile([7, 14], f32, tag="Rt")
                    nc.tensor.matmul(Rt, c_bpat, c_bias, start=True, stop=False)
                    for dw in range(3):
                        nc.tensor.matmul(
                            Rt, V[:, 7 * dw:7 * dw + 7],
                            c_f[:, CF_BLK + 14 * dw:CF_BLK + 14 * dw + 14],
                            start=False, stop=(dw == 2))
                    # Et cols 7:32 are never written: the tg matmul's output
                    # rows 7:32 are garbage that no one reads
                    Et = etp.tile([7, 39], f32r, tag="Et")
                    nc.scalar.copy(Et[:, 0:7], Rt[:, 0:7])
                    nc.scalar.copy(Et[:, 32:39], Rt[:, 7:14])
                    return Et

                def stage_out(b, Et):
                    # tg for BOTH channels in one matmul + ONE wide copy; out
                    # rows via A^T against tg in 256-row halves, two matmuls
                    # filling a 2-bank PSUM tile drained by ONE wide ACT
                    # copy; one 512 KB sync-ring store per (co, half)
                    tg = tgpool.tile([7, CO, 512], bf16, tag="tg")
                    tg_ps = ptg.tile([39, 512], f32, tag="tg_ps")
                    nc.tensor.matmul(tg_ps, Et, c_la, start=True, stop=True)
                    for co in range(CO):
                        nc.scalar.copy(tg[:, co, :],
                                       tg_ps[32 * co:32 * co + 7, :])
                    for co in range(CO):
                        for g in range(2):
                            ot = opool.tile([128, 2, 512], f32, tag="ot")
                            oc_ps = poc.tile([128, 2, 512], f32, tag="oc")
                            for tt in range(2):
                                nc.tensor.matmul(
                                    oc_ps[:, tt, :],
                                    c_atr[:, 256 * g + 128 * tt:
                                          256 * g + 128 * (tt + 1)],
                                    tg[:, co, :],
                                    start=True, stop=True)
                            # ALL wide copies on ACT: DVE must stay free for
                            # the tree+V chain (it is the engine every batch
                            # couples through); ALL stores ride the sync
                            # ring — a store on the SWDGE queue head-of-line
                            # blocks later batches' load descriptor gen
                            nc.scalar.copy(ot, oc_ps)
                            nc.sync.dma_start(
                                out=y[b, co, 256 * g:256 * (g + 1), :]
                                    .rearrange("(p tt) w -> p tt w", tt=2),
                                in_=ot,
                            )

                for b in range(BLOC):
                    st = load_colsum(b)
                    # the small V/R ops outrank the next batch's tree adds in
                    # the DVE/PE streams (the scheduler's optimistic DMA model
                    # otherwise slots tree(b+1) first, delaying V by ~4.5 us);
                    # the heavy out-chain keeps natural priority
                    with tc.high_priority():
                        V = stage_v(b, st)
                        Et = stage_r(b, V)
                    stage_out(b, Et)

            for _ in range(N_REPS):
                emit_once()
    return nc


def _get_prog():
    global _PROG
    if _PROG is None:
        _PROG = build_program()
        _PROG.finalize()
    return _PROG


# ---------------------------------------------------------------------------
# host entry point
# ---------------------------------------------------------------------------

def _per_core_inputs(x, consts):
    in_maps = []
    for c in range(NCORES):
        xc = np.ascontiguousarray(x[c * BLOC:(c + 1) * BLOC])
        xe = np.empty((BLOC, 2, 4, CIN, W), dtype=np.float32)
        xe[:, 0] = xc[:, :, 0:4, :].transpose(0, 2, 1, 3)
        xe[:, 1] = xc[:, :, H - 4:H, :].transpose(0, 2, 1, 3)
        in_maps.append({
            "xs": xc.reshape(BLOC, 128, 16, W),
            "xe": xe,
            **consts,
        })
    return in_maps


def kernel(x, g_w, g_b, theta_w, theta_b):
    global LAST_EXEC_NS, LAST_TRACE_PATH
    from concourse.bass_utils import run_bass_kernel_spmd

    x = np.ascontiguousarray(np.asarray(x, dtype=np.float32))
    g_w = np.asarray(g_w, dtype=np.float32)
    g_b = np.asarray(g_b, dtype=np.float32)
    theta_w = np.asarray(theta_w, dtype=np.float32)
    theta_b = np.asarray(theta_b, dtype=np.float32)

    consts = build_consts(g_w, g_b, theta_w, theta_b)
    nc = _get_prog()
    in_maps = _per_core_inputs(x, consts)
    try:
        res = run_bass_kernel_spmd(nc, in_maps, core_ids=list(range(NCORES)),
                                   trace=TRACE)
    except ModuleNotFoundError:
        # no NTFF profiling hook in this environment; run untraced
        res = run_bass_kernel_spmd(nc, in_maps, core_ids=list(range(NCORES)),
                                   trace=False)
    LAST_EXEC_NS = res.exec_time_ns
    if TRACE and res.instructions_and_trace is not None:
        LAST_TRACE_PATH = res.instructions_and_trace[1]
    return np.concatenate([res.results[c]["y"] for c in range(NCORES)], axis=0)
